# revision 1
# baseline (speedup 1.0000x reference)
"""Trainium2 Bass kernel for a 2-layer GCN (BayesianGCN in eval mode).

Math: with dinv = rsqrt(in_degree + 2), x' = dinv*x (pre-scaled on host):

    agg1[d] = sum_{e: dst=d} x'[src_e] + 2*x'[d]          (gather + one-hot matmul)
    u       = relu((dinv[d]*agg1) @ W1 + b1)
    h2'     = dinv * (u @ W2)                             (64-wide, AllGathered)
    agg2[d] = sum_{e: dst=d} h2'[src_e] + 2*h2'[d]
    out     = log_softmax(dinv[d]*agg2[d] + b2)

Distribution: nodes (rows / dst segments) sharded over 8 cores.  Layer 1
gathers rows of the pre-scaled fp16 x table; layer 2 exchanges the 64-wide
h2' table with a chunked AllGather and gathers 256-B rows each
packing TWO nodes; edge streams are parity-sorted so each 128-edge batch
reads a single 64-col half of its gathered rows.  The dense W1/relu/W2/h2
pipeline runs per 128-row block so AllGather chunks launch DURING layer-1
aggregation, interleaved between gather calls on the Pool engine; layer-2
pair table (25k rows) fits one int16 index stream, so layer 2 needs
no A/B table split.

Per-edge aggregation: edges sorted by dst, padded per 128-dst block; row
gathers via SWDGE dma_gather (1024 idxs/call, the deployed-ucode cap);
segment sums as one-hot matmuls accumulated per dst-block in PSUM.  The
one-hot M matrices are generated ON-CHIP by the vector engine in a
transposed [pos, dst, batch] layout (trailing-stride-1 keeps DVE 2x mode):
M = is_equal(IOTA, ldst).  No M matrices are streamed from HBM and no
per-edge scale is applied (the norm lives in the tables).  Self-loops
enter each block as a (2*I) matmul; b2 enters layer-2 PSUM as a
rank-reduced (ones/128 @ b2_rep) matmul so the PSUM->SBUF move is a pure
per-partition scale on the scalar engine.  log_softmax batches all Exp
passes then runs ONE Ln over [128, T] to avoid activation-table reloads.

Host preprocessing is graph-index work only: degrees, rsqrt normalizers,
edge sorting/padding, int16 gather indices (the layer-1 table is split
in two <32k-row halves), local-dst streams, and layer-2 pair-row/parity
bookkeeping.
"""

import os
import sys

import numpy as np

sys.path.insert(0, "/opt/trn_rl_repo")

import concourse.bacc as bacc  # noqa: E402
import concourse.bass as bass  # noqa: E402
from concourse import mybir  # noqa: E402
from concourse.bass_utils import run_bass_kernel_spmd  # noqa: E402
from concourse.library_config import mlp as _mlp_lib  # noqa: E402

F32 = mybir.dt.float32
F16 = mybir.dt.float16
I16 = mybir.dt.int16
ALU = mybir.AluOpType
ACT = mybir.ActivationFunctionType
AX = mybir.AxisListType

N = 50000
E = 800000
DIN = 128
H = 128
C = 64
NCORES = 8
GPOS = int(os.environ.get("K_GPOS", 1024))   # idxs per dma_gather (ucode cap)
GSLOTS = int(os.environ.get("K_GSLOTS", 12))  # gather buffer slots per half
CH = int(os.environ.get("K_CH", 1))          # AllGather chunks
RB1 = int(os.environ.get('K_RB1', 2))   # layer-1 M rotation depth (blocks)
RB2 = int(os.environ.get('K_RB2', 4))   # layer-2 M rotation depth (blocks)
PAD_DST = 200.0  # ldst padding value (never matches iota 0..127)
DMASCRATCH = int(os.environ.get("K_DMASCRATCH", 16384))
NSWQ = int(os.environ.get("K_NSWQ", 1))      # SWDGE queues for gathers


def _shard_sizes(n):
    shard = n // NCORES
    half = n // 2
    t = (shard + 127) // 128
    return shard, half, t, t * 128


# ----------------------------------------------------------------------------
# Host preprocessing (graph-index work only).
# ----------------------------------------------------------------------------

def _pack_gidx(gflat):
    return np.tile(np.ascontiguousarray(gflat.reshape(-1, 16).T), (8, 1))


def _chunk_starts(shard, t):
    """Block-aligned AllGather chunk row boundaries: CB blocks per chunk."""
    cb = (t + CH - 1) // CH
    starts = [min(c * cb * 128, shard) for c in range(CH + 1)]
    return cb, starts


def _preprocess(edge_index, n):
    """Build per-core edge streams.

    Layer 1: per (core, half-by-src) streams sorted by local dst, padded per
    128-dst block to a uniform NB1[h] batches.
    Layer 2: same but indexed by pair-row in the chunked AllGather layout,
    split at the chunk-aligned PA boundary, sorted [even | odd] node-row
    parity with uniform KE/KO batch split.

    Returns (dinv, cores list of dicts, cfg tuple)."""
    shard, half, T, shard_pad = _shard_sizes(n)
    cb, cstart = _chunk_starts(shard, T)
    src = np.asarray(edge_index[0], dtype=np.int64)
    dst = np.asarray(edge_index[1], dtype=np.int64)
    deg = np.bincount(dst, minlength=n).astype(np.float32) + 2.0
    dinv = (1.0 / np.sqrt(deg)).astype(np.float32)

    order = np.argsort(dst, kind="stable")
    ssrc = src[order]
    sdst = dst[order]
    core_bnd = np.searchsorted(sdst, np.arange(NCORES + 1) * shard)

    # layer-2 table-row mapping (chunk-major AllGather layout, node pairs)
    k2 = ssrc // shard
    r2 = ssrc % shard
    cs_arr = np.asarray(cstart, np.int64)
    chk = np.clip(np.searchsorted(cs_arr, r2, side="right") - 1, 0, CH - 1)
    szs = cs_arr[chk + 1] - cs_arr[chk]
    prefix = np.concatenate([[0], np.cumsum(np.diff(cs_arr) * NCORES)])
    noderow = prefix[chk] + k2 * szs + (r2 - cs_arr[chk])
    pairrow = noderow >> 1
    odd = (noderow & 1).astype(np.int64)
    # pair table has n//2 (< 32767) rows: ONE int16-indexable layer-2 stream

    # ---- pass 1: per-core/half/block counts -> uniform NB1, KE, KO -------
    percore = []
    NB1 = [1, 1]
    KE = 1
    KO = 1
    for k in range(NCORES):
        lo, hi = core_bnd[k], core_bnd[k + 1]
        cs = ssrc[lo:hi]
        dl = sdst[lo:hi] - k * shard
        i2 = pairrow[lo:hi].astype(np.int16)
        o2 = odd[lo:hi]
        halves = []
        for h in (0, 1):
            m1 = (cs >= half) == (h == 1)
            d1 = dl[m1]
            i1 = (cs[m1] - h * half).astype(np.int16)
            b1 = np.searchsorted(d1, np.arange(T + 1) * 128)
            c1 = np.diff(b1)
            if len(c1):
                NB1[h] = max(NB1[h], int((c1.max() + 127) // 128))
            halves.append((d1, i1, b1))
        b2 = np.searchsorted(dl, np.arange(T + 1) * 128)
        for b in range(T):
            s, e = int(b2[b]), int(b2[b + 1])
            ne = int((o2[s:e] == 0).sum())
            no = (e - s) - ne
            KE = max(KE, (ne + 127) // 128)
            KO = max(KO, (no + 127) // 128)
        percore.append((halves, (dl, i2, o2, b2)))
    cfg = (NB1[0], NB1[1], KE, KO)

    # ---- pass 2: build padded streams ------------------------------------
    cores = []
    for k in range(NCORES):
        d = {}
        halves, (d2, i2, o2, b2) = percore[k]
        for h, nm in ((0, "A"), (1, "B")):
            d1, i1, b1 = halves[h]
            nb1 = NB1[h]
            tot1 = T * nb1 * 128
            g1 = np.zeros(tot1, np.int16)
            l1 = np.full(tot1, PAD_DST, np.float16)
            for b in range(T):
                s, e = int(b1[b]), int(b1[b + 1])
                pos = b * nb1 * 128 + np.arange(e - s)
                g1[pos] = i1[s:e]
                l1[pos] = (d1[s:e] - b * 128).astype(np.float16)
            d["gidx1" + nm] = _pack_gidx(g1)
            d["ldst1" + nm] = np.ascontiguousarray(l1.reshape(-1, 128).T)

        nb2 = KE + KO
        tot2 = T * nb2 * 128
        g2 = np.zeros(tot2, np.int16)
        l2 = np.full(tot2, PAD_DST, np.float16)
        for b in range(T):
            s, e = int(b2[b]), int(b2[b + 1])
            ob = o2[s:e]
            for par, off in ((0, 0), (1, KE * 128)):
                mp = ob == par
                cnt = int(mp.sum())
                pos = b * nb2 * 128 + off + np.arange(cnt)
                g2[pos] = i2[s:e][mp]
                l2[pos] = (d2[s:e][mp] - b * 128).astype(np.float16)
        d["gidx2"] = _pack_gidx(g2)
        d["ldst2"] = np.ascontiguousarray(l2.reshape(-1, 128).T)
        cores.append(d)
    return dinv, cores, cfg


# ----------------------------------------------------------------------------
# Bass kernel.
# ----------------------------------------------------------------------------

def _build(n, cfg):
    shard, half, T, shard_pad = _shard_sizes(n)
    cb, cstart = _chunk_starts(shard, T)
    NB1 = {"A": cfg[0], "B": cfg[1]}
    KE, KO = cfg[2], cfg[3]
    NB2 = KE + KO
    NBMAX = max(NB1["A"], NB1["B"], NB2)

    tot1 = {h: T * NB1[h] * 128 for h in "AB"}
    tot2 = T * NB2 * 128
    NG1 = {h: (tot1[h] + GPOS - 1) // GPOS for h in "AB"}
    NG2 = (tot2 + GPOS - 1) // GPOS
    NG1MAX = max(NG1.values())

    def npos1(h, g):
        return min(GPOS, tot1[h] - g * GPOS)

    def npos2(g):
        return min(GPOS, tot2 - g * GPOS)

    def bmax1(h, g):
        return min(T - 1, ((g + 1) * GPOS - 1) // (NB1[h] * 128))

    def bmax2(g):
        return min(T - 1, ((g + 1) * GPOS - 1) // (NB2 * 128))

    def gneed1(h, b):
        return ((b + 1) * NB1[h] * 128 - 1) // GPOS

    def gneed2(b):
        return ((b + 1) * NB2 * 128 - 1) // GPOS

    def cblocks(c):
        # number of row-blocks whose ccin writes feed chunk c
        return (cstart[c + 1] - cstart[c] + 127) // 128

    LAG1, LAG2 = 2, 4   # dense-pipeline lags (blocks) behind aggregation
    SLACK = 6           # blocks of slack before an interleaved chunk issue

    nc = bacc.Bacc(None, target_bir_lowering=False, num_devices=NCORES,
                   dynamic_dma_scratch_size=DMASCRATCH,
                   num_swdge_queues=NSWQ)

    # ---- I/O -------------------------------------------------------------
    xlo = nc.declare_dram_parameter("xlo", [half, DIN], F16, isOutput=False)
    xhi = nc.declare_dram_parameter("xhi", [half, DIN], F16, isOutput=False)
    xown = nc.declare_dram_parameter("xown", [shard_pad, DIN], F16, isOutput=False)
    gidx1, ldst1 = {}, {}
    for h in "AB":
        gidx1[h] = nc.declare_dram_parameter(
            f"gidx1{h}", [128, tot1[h] // 16], I16, isOutput=False)
        ldst1[h] = nc.declare_dram_parameter(
            f"ldst1{h}", [128, T * NB1[h]], F16, isOutput=False)
    gidx2 = nc.declare_dram_parameter(
        "gidx2", [128, tot2 // 16], I16, isOutput=False)
    ldst2 = nc.declare_dram_parameter(
        "ldst2", [128, T * NB2], F16, isOutput=False)
    dinvown = nc.declare_dram_parameter("dinvown", [128, T], F32, isOutput=False)
    dinvrep = nc.declare_dram_parameter("dinvrep", [128, shard_pad], F16, isOutput=False)
    w1 = nc.declare_dram_parameter("w1", [DIN, H], F16, isOutput=False)
    w2 = nc.declare_dram_parameter("w2", [H, C], F16, isOutput=False)
    b1 = nc.declare_dram_parameter("b1", [H, 1], F32, isOutput=False)
    b2r = nc.declare_dram_parameter("b2r", [128, C], F16, isOutput=False)
    twoI = nc.declare_dram_parameter("twoI", [128, 128], F16, isOutput=False)
    oneN = nc.declare_dram_parameter("oneN", [128, 128], F16, isOutput=False)
    iota = nc.declare_dram_parameter("iota", [128, 128 * NBMAX], F16, isOutput=False)
    out = nc.declare_dram_parameter("out", [shard_pad, C], F32, isOutput=True)

    # ---- internal DRAM ---------------------------------------------------
    ccin = nc.dram_tensor("ccin", [shard, C], F16)
    h2full = nc.dram_tensor("h2full", [n // 2, 128], F16, addr_space="Shared")

    # ---- SBUF ------------------------------------------------------------
    A = nc.alloc_sbuf_tensor
    gidx1_sb = {h: A(f"gidx1{h}_sb", [128, tot1[h] // 16], I16) for h in "AB"}
    gidx2_sb = A("gidx2_sb", [128, tot2 // 16], I16)
    ldst1_sb = {h: A(f"ldst1{h}_sb", [128, T * NB1[h]], F16) for h in "AB"}
    ldst2_sb = A("ldst2_sb", [128, T * NB2], F16)
    G = {h: A(f"g{h}", [128, GSLOTS * GPOS], F16) for h in "AB"}
    M1 = {h: A(f"m1{h}", [128, RB1 * 128 * NB1[h]], F16) for h in "AB"}
    M2 = A("m2", [128, RB2 * 128 * NB2], F16)
    accT = A("accT", [128, shard_pad], F16)
    uT = A("uT", [128, shard_pad], F16)
    h2p = A("h2p", [128, T * C], F16)
    qmB = A("qmB", [128, T * C], F32)
    dinvrep_sb = A("dinvrep_sb", [128, shard_pad], F16)
    dvo_sb = A("dvo_sb", [128, T], F32)
    w1_sb = A("w1_sb", [DIN, H], F16)
    w2_sb = A("w2_sb", [H, C], F16)
    b1_sb = A("b1_sb", [H, 1], F32)
    b2r_sb = A("b2r_sb", [128, C], F16)
    twoI_sb = A("twoI_sb", [128, 128], F16)
    oneN_sb = A("oneN_sb", [128, 128], F16)
    iota_sb = A("iota_sb", [128, 128 * NBMAX], F16)
    xt = [A(f"xt{i}", [128, DIN], F16) for i in range(3)]
    qoAll = A("qoAll", [128, T * C], F32)
    nmxB = A("nmxB", [128, T], F32)
    smeB = A("smeB", [128, T], F32)
    lnsB = A("lnsB", [128, T], F32)
    qe = A("qe", [128, C], F16)

    pm1 = [nc.alloc_psum_tensor(f"pm1{i}", [128, 128], F32) for i in (0, 1)]
    mmP = [nc.alloc_psum_tensor(f"mmP{i}", [128, 128], F32) for i in (0, 1)]
    h2P = [nc.alloc_psum_tensor(f"h2P{i}", [128, C], F32) for i in (0, 1)]
    pm2 = [nc.alloc_psum_tensor(f"pm2{i}", [128, C], F32) for i in (0, 1)]

    def gview(h, g, npos):
        base = (g % GSLOTS) * GPOS
        return G[h][:, base: base + npos].rearrange("p (s e) -> p s e", e=128)

    def g_batch(h, q):
        g = q // (GPOS // 128)
        base = (g % GSLOTS) * GPOS + (q % (GPOS // 128)) * 128
        return G[h][:, base: base + 128]

    def iota_view(nb):
        return iota_sb[:].rearrange("p (d s) -> p d s", s=NBMAX)[:, :, :nb]

    def m1_slot(h, b):
        nb = NB1[h]
        base = (b % RB1) * 128 * nb
        return M1[h][:, base: base + 128 * nb].rearrange(
            "p (d s) -> p d s", s=nb)

    def m2_slot(b):
        base = (b % RB2) * 128 * NB2
        return M2[:, base: base + 128 * NB2].rearrange(
            "p (d s) -> p d s", s=NB2)

    def ldst_bcast(t_sb, b, nb):
        return t_sb[:, b * nb: (b + 1) * nb].to_broadcast(
            [128, nb, 128]).rearrange("p s d -> p d s")

    # ---- static schedules (1 sem inc per instruction on ve/pe/ac) -------
    # PE emission: per-iteration [agg i, w1 i-LAG1, w2 i-LAG2], then L2
    # blocks (self-loop + b2 matmul + batches).
    BL1 = 1 + NB1["A"] + NB1["B"]
    BL2 = 2 + NB2
    pe_blk1, pe_w1, pe_h2, pe_blk2 = {}, {}, {}, {}
    _p = 0
    for i in range(T + LAG2):
        if i < T:
            _p += BL1
            pe_blk1[i] = _p
        if 0 <= i - LAG1 < T:
            _p += 1
            pe_w1[i - LAG1] = _p
        if 0 <= i - LAG2 < T:
            _p += 1
            pe_h2[i - LAG2] = _p
    for b in range(T):
        _p += BL2
        pe_blk2[b] = _p
    PE_END = _p

    # ACT emission: [copy i, relu i-LAG1, h2c i-LAG2], then [qm b, exp b-2],
    # exp tail, one Ln.
    ac_copy, ac_relu, ac_h2, ac_qm, ac_exp = {}, {}, {}, {}, {}
    _a = 0
    for i in range(T + LAG2):
        if i < T:
            _a += 1
            ac_copy[i] = _a
        if 0 <= i - LAG1 < T:
            _a += 1
            ac_relu[i - LAG1] = _a
        if 0 <= i - LAG2 < T:
            _a += 1
            ac_h2[i - LAG2] = _a
    EXLAG = 2
    SPLITB = max(1, T - 5)  # early log-softmax drain boundary
    AC_LN1 = None
    for i in range(T + EXLAG):
        if i < T:
            _a += 1
            ac_qm[i] = _a
        if 0 <= i - EXLAG < T:
            _a += 1
            ac_exp[i - EXLAG] = _a
            if i - EXLAG == SPLITB - 1:
                _a += 1
                AC_LN1 = _a
    _a += 1
    AC_LN2 = _a

    # VE emission: [m1 b, accm b-1], accm tail, m2 pre, [negmax b, m2 b+RB2],
    # out tail.
    ve_m1, ve_acc, ve_m2, ve_negmax, ve_out = {}, {}, {}, {}, {}
    _v = 0
    for b in range(T):
        _v += 2
        ve_m1[b] = _v
        if b >= 1:
            _v += 1
            ve_acc[b - 1] = _v
    _v += 1
    ve_acc[T - 1] = _v
    for b in range(min(RB2, T)):
        _v += 1
        ve_m2[b] = _v
    for b in range(T):
        _v += 1
        ve_negmax[b] = _v
        if b + RB2 < T:
            _v += 1
            ve_m2[b + RB2] = _v
    for b in range(T):
        _v += 1
        ve_out[b] = _v
    VE_END = _v

    # interleaved chunk-issue points within the L1 gather block sequence
    chunk_after = {}
    for c in range(CH):
        blk = min(T - 1, cb * (c + 1) + SLACK)
        chunk_after.setdefault(blk, []).append(c)
    # staged preloads: ld_a gates L1 gathers, ld_b1 gates M-gen/aggregation
    # start, ld_b2 gates the dense pipeline, ld_c gates L2.
    LD_A, LD_B1, LD_B2, LD_C = 2 * 16, 4 * 16, 7 * 16, 2 * 16

    GV = {}
    gcnt = {(h, sl): 0 for h in "AB" for sl in range(GSLOTS)}
    counters = {}

    def mk_counter(name):
        counters[name] = 0

        def bump(inst, sem_h, d):
            counters[name] += d
            inst.then_inc(sem_h, d)
            return counters[name]

        return bump

    def rows(t):
        r0 = t * 128
        return r0, min(r0 + 128, shard)

    from contextlib import ExitStack

    with ExitStack() as _st:
        block = _st.enter_context(nc.Block())
        sem = lambda nm: _st.enter_context(nc.semaphore(nm))
        ld_a = sem("ld_a")
        ld_a2 = sem("ld_a2")
        ld_b1 = sem("ld_b1")
        ld_b2 = sem("ld_b2")
        ld_c = sem("ld_c")
        w_cc = [sem(f"w_cc{c}") for c in range(CH)]
        xq = [sem(f"xq{i}") for i in range(3)]
        w_out = sem("w_out")
        gq = {h: [sem(f"g{h}{i}") for i in range(GSLOTS)] for h in "AB"}
        ve = sem("ve")
        pe = sem("pe")
        ac = sem("ac")
        cc = sem("cc")

        # --------------------------------------------------------- gpsimd
        @block.gpsimd
        def _(gp: bass.BassGpSimd):
            gp.load_library(_mlp_lib)
            gp.wait_ge(ld_a, LD_A)

            def issue_chunk(c, po):
                gp.wait_ge(w_cc[c], 16 * cblocks(c))
                sz = cstart[c + 1] - cstart[c]
                gp.collective_compute(
                    "AllGather",
                    ALU.bypass,
                    replica_groups=[list(range(NCORES))],
                    ins=[ccin[cstart[c]: cstart[c + 1], :]],
                    outs=[h2full[po: po + NCORES * sz // 2, :]],
                ).then_inc(cc, 1)
                return po + NCORES * sz // 2

            qctr = [0]

            def gather(li, h, g, npos, tab, idx_sb):
                gcnt[(h, g % GSLOTS)] += 16
                GV[(li, g, h)] = gcnt[(h, g % GSLOTS)]
                qctr[0] += 1
                gp.dma_gather(
                    out_ap=gview(h, g, npos),
                    in_ap=tab,
                    idxs_ap=idx_sb[
                        :, g * GPOS // 16: g * GPOS // 16 + npos // 16],
                    num_idxs=npos,
                    num_idxs_reg=npos,
                    elem_size=128,
                    queue_num=qctr[0] % NSWQ,
                ).then_inc(gq[h][g % GSLOTS], 16)

            po = 0
            issued = 0
            # layer-1 gathers, emitted in PE consumption order (by block),
            # with AllGather chunks interleaved at their readiness points.
            gw = {h: -1 for h in "AB"}
            lda2_waited = False
            for b in range(T):
                for h in "AB":
                    while gw[h] < gneed1(h, b):
                        gw[h] += 1
                        g = gw[h]
                        if g >= GSLOTS:
                            if not lda2_waited:
                                gp.wait_ge(ld_a2, 32)
                                lda2_waited = True
                            gp.wait_ge(pe, pe_blk1[bmax1(h, g - GSLOTS)])
                        gather(0, h, g, npos1(h, g),
                               (xlo if h == "A" else xhi)[:], gidx1_sb[h])
                for c in chunk_after.get(b, []):
                    po = issue_chunk(c, po)
                    issued += 1
            while issued < CH:
                po = issue_chunk(issued, po)
                issued += 1
            # layer-2 gathers: one int16 stream over the whole pair table,
            # in PE consumption order; G["A"] slots are reused.
            gp.wait_ge(ld_c, LD_C)
            gp.wait_ge(cc, CH)
            gp.wait_ge(pe, pe_blk1[T - 1])  # L1 done: G slots free
            g2w = -1
            for b in range(T):
                while g2w < gneed2(b):
                    g2w += 1
                    g = g2w
                    if g >= GSLOTS:
                        gp.wait_ge(pe, pe_blk2[bmax2(g - GSLOTS)])
                    gather(1, "A", g, npos2(g), h2full[:], gidx2_sb)

        # ----------------------------------------------------------- sync
        @block.sync
        def _(sp: bass.BassEngine):
            gcols = GSLOTS * GPOS // 16  # idx cols for the first GSLOTS rounds
            for h in "AB":
                hc = min(gcols, tot1[h] // 16)
                sp.dma_start(out=gidx1_sb[h][:, :hc],
                             in_=gidx1[h][:, :hc]).then_inc(ld_a, 16)
            for h in "AB":
                hc = min(gcols, tot1[h] // 16)
                if tot1[h] // 16 > hc:
                    sp.dma_start(out=gidx1_sb[h][:, hc:],
                                 in_=gidx1[h][:, hc:]).then_inc(ld_a2, 16)
                else:
                    sp.nop().then_inc(ld_a2, 16)
            early1 = [
                (ldst1_sb["A"][:], ldst1["A"][:]),
                (ldst1_sb["B"][:], ldst1["B"][:]),
                (iota_sb[:], iota[:]), (twoI_sb[:], twoI[:]),
            ]
            assert len(early1) * 16 == LD_B1
            for o_, i_ in early1:
                sp.dma_start(out=o_, in_=i_).then_inc(ld_b1, 16)
            early2 = [
                (dvo_sb[:], dinvown[:]), (dinvrep_sb[:], dinvrep[:]),
                (w1_sb[:], w1[:]), (b1_sb[:], b1[:]), (w2_sb[:], w2[:]),
                (b2r_sb[:], b2r[:]), (oneN_sb[:], oneN[:]),
            ]
            assert len(early2) * 16 == LD_B2
            for o_, i_ in early2:
                sp.dma_start(out=o_, in_=i_).then_inc(ld_b2, 16)
            late = [
                (gidx2_sb[:], gidx2[:]),
                (ldst2_sb[:], ldst2[:]),
            ]
            assert len(late) * 16 == LD_C
            CCLAG = 7  # ccin trails xt loads; keeps SP in-order emission live
            for t in range(T + CCLAG):
                if t < T:
                    if 3 <= t < 3 + len(late):
                        o_, i_ = late[t - 3]
                        sp.dma_start(out=o_, in_=i_).then_inc(ld_c, 16)
                    if t >= 3:
                        sp.wait_ge(pe, pe_blk1[t - 3])  # WAR xt slot
                    sp.dma_start(
                        out=xt[t % 3][:], in_=xown[t * 128: (t + 1) * 128, :]
                    ).then_inc(xq[t % 3], 16)
                tc = t - CCLAG
                if 0 <= tc < T:
                    r0, r1 = rows(tc)
                    sp.wait_ge(ac, ac_h2[tc])
                    sp.dma_start(
                        out=ccin[r0:r1, :],
                        in_=h2p[: r1 - r0, tc * C: (tc + 1) * C],
                    ).then_inc(w_cc[min(tc // cb, CH - 1)], 16)
            sp.wait_ge(ve, ve_out[SPLITB - 1])
            sp.dma_start(
                out=out[: SPLITB * 128].rearrange("(t p) c -> p t c", p=128),
                in_=qoAll[:, : SPLITB * C].rearrange("p (t c) -> p t c", c=C),
            ).then_inc(w_out, 16)
            sp.wait_ge(ve, VE_END)
            sp.dma_start(
                out=out[SPLITB * 128:].rearrange("(t p) c -> p t c", p=128),
                in_=qoAll[:, SPLITB * C:].rearrange("p (t c) -> p t c", c=C),
            ).then_inc(w_out, 16)
            sp.wait_ge(w_out, 32)

        # --------------------------------------------------------- vector
        @block.vector
        def _(vec: bass.BassVectorEngine):
            bump = mk_counter("ve")

            def vinc(inst):
                return bump(inst, ve, 1)

            vec.wait_ge(ld_b1, LD_B1)
            b2_waited = [False]

            def gen_m1(b):
                for h in "AB":
                    if b >= RB1:
                        vec.wait_ge(pe, pe_blk1[b - RB1])  # WAR M1 slot
                    vinc(vec.tensor_tensor(
                        out=m1_slot(h, b), in0=iota_view(NB1[h]),
                        in1=ldst_bcast(ldst1_sb[h], b, NB1[h]),
                        op=ALU.is_equal,
                    ))
                assert counters["ve"] == ve_m1[b]

            def gen_m2(b):
                vinc(vec.tensor_tensor(
                    out=m2_slot(b), in0=iota_view(NB2),
                    in1=ldst_bcast(ldst2_sb, b, NB2),
                    op=ALU.is_equal,
                ))
                assert counters["ve"] == ve_m2[b]

            def acc_scale(t):
                if not b2_waited[0]:
                    vec.wait_ge(ld_b2, LD_B2)
                    b2_waited[0] = True
                vec.wait_ge(ac, ac_copy[t])
                sl = slice(t * 128, (t + 1) * 128)
                vinc(vec.tensor_tensor(
                    out=accT[:, sl], in0=accT[:, sl],
                    in1=dinvrep_sb[:, sl], op=ALU.mult,
                ))
                assert counters["ve"] == ve_acc[t]

            for b in range(T):
                gen_m1(b)
                if b >= 1:
                    acc_scale(b - 1)
            acc_scale(T - 1)
            vec.wait_ge(ld_c, LD_C)
            for b in range(min(RB2, T)):
                gen_m2(b)
            for b in range(T):
                vec.wait_ge(ac, ac_qm[b])
                vinc(vec.tensor_reduce(
                    out=nmxB[:, b: b + 1], in_=qmB[:, b * C: (b + 1) * C],
                    axis=AX.X, op=ALU.max, negate=True,
                ))
                assert counters["ve"] == ve_negmax[b]
                if b + RB2 < T:
                    vec.wait_ge(pe, pe_blk2[b])  # WAR M2 slot b+RB2
                    gen_m2(b + RB2)
            vec.wait_ge(ac, AC_LN1)
            for b in range(T):
                if b == SPLITB:
                    vec.wait_ge(ac, AC_LN2)
                vinc(vec.scalar_tensor_tensor(
                    out=qoAll[:, b * C: (b + 1) * C],
                    in0=qmB[:, b * C: (b + 1) * C],
                    scalar=lnsB[:, b: b + 1],
                    in1=nmxB[:, b: b + 1].to_broadcast([128, C]),
                    op0=ALU.subtract, op1=ALU.add,
                ))
                assert counters["ve"] == ve_out[b]
            assert counters["ve"] == VE_END

        # --------------------------------------------------------- tensor
        @block.tensor
        def _(te: bass.BassTensorEngine):
            bump = mk_counter("pe")

            def pinc(inst):
                return bump(inst, pe, 1)

            te.wait_ge(ld_b1, LD_B1)
            peb2 = [False]
            gwaited = {h: -1 for h in "AB"}

            def agg_block(b):
                if b >= 2:
                    te.wait_ge(ac, ac_copy[b - 2])  # WAR pm1 slot
                te.wait_ge(xq[b % 3], 16 * (b // 3 + 1))
                te.wait_ge(ve, ve_m1[b])
                for h in "AB":
                    while gwaited[h] < gneed1(h, b):
                        gwaited[h] += 1
                        g = gwaited[h]
                        te.wait_ge(gq[h][g % GSLOTS], GV[(0, g, h)])
                pinc(te.matmul(
                    out=pm1[b % 2][:], lhsT=xt[b % 3][:], rhs=twoI_sb[:],
                    start=True, stop=False,
                ))
                for h in "AB":
                    nb = NB1[h]
                    for j in range(nb):
                        pinc(te.matmul(
                            out=pm1[b % 2][:],
                            lhsT=g_batch(h, b * nb + j),
                            rhs=m1_slot(h, b)[:, :, j],
                            start=False,
                            stop=(h == "B" and j == nb - 1),
                        ))
                assert counters["pe"] == pe_blk1[b]

            def w1_mm(t):
                if not peb2[0]:
                    te.wait_ge(ld_b2, LD_B2)
                    peb2[0] = True
                te.wait_ge(ve, ve_acc[t])
                if t >= 2:
                    te.wait_ge(ac, ac_relu[t - 2])  # WAR mmP slot
                pinc(te.matmul(
                    out=mmP[t % 2][:], lhsT=w1_sb[:],
                    rhs=accT[:, t * 128: (t + 1) * 128],
                    start=True, stop=True,
                ))
                assert counters["pe"] == pe_w1[t]

            def w2_mm(t):
                te.wait_ge(ac, ac_relu[t])
                if t >= 2:
                    te.wait_ge(ac, ac_h2[t - 2])  # WAR h2P slot
                pinc(te.matmul(
                    out=h2P[t % 2][:], lhsT=uT[:, t * 128: (t + 1) * 128],
                    rhs=w2_sb[:], start=True, stop=True,
                ))
                assert counters["pe"] == pe_h2[t]

            for i in range(T + LAG2):
                if i < T:
                    agg_block(i)
                if 0 <= i - LAG1 < T:
                    w1_mm(i - LAG1)
                if 0 <= i - LAG2 < T:
                    w2_mm(i - LAG2)
            te.wait_ge(ld_c, LD_C)
            g2waited = -1
            for b in range(T):
                if b >= 2:
                    te.wait_ge(ac, ac_qm[b - 2])  # WAR pm2 slot
                te.wait_ge(ac, ac_h2[b])
                te.wait_ge(ve, ve_m2[b])
                while g2waited < gneed2(b):
                    g2waited += 1
                    g = g2waited
                    te.wait_ge(gq["A"][g % GSLOTS], GV[(1, g, "A")])
                pinc(te.matmul(
                    out=pm2[b % 2][:], lhsT=twoI_sb[:],
                    rhs=h2p[:, b * C: (b + 1) * C], start=True, stop=False,
                ))
                pinc(te.matmul(
                    out=pm2[b % 2][:], lhsT=oneN_sb[:], rhs=b2r_sb[:],
                    start=False, stop=False,
                ))
                for j in range(NB2):
                    ebit = 0 if j < KE else 1
                    pinc(te.matmul(
                        out=pm2[b % 2][:],
                        lhsT=m2_slot(b)[:, :, j],
                        rhs=g_batch("A", b * NB2 + j)[
                            :, ebit * C: (ebit + 1) * C],
                        start=False,
                        stop=(j == NB2 - 1),
                    ))
                assert counters["pe"] == pe_blk2[b]
            assert counters["pe"] == PE_END

        # --------------------------------------------------------- scalar
        @block.scalar
        def _(sc: bass.BassScalarEngine):
            bump = mk_counter("ac")

            def ainc(inst):
                return bump(inst, ac, 1)

            sc.wait_ge(ld_b1, LD_B1)
            acb2 = [False]
            for i in range(T + LAG2):
                if i < T:
                    sc.wait_ge(pe, pe_blk1[i])
                    ainc(sc.activation(
                        out=accT[:, i * 128: (i + 1) * 128],
                        in_=pm1[i % 2][:], func=ACT.Copy,
                    ))
                    assert counters["ac"] == ac_copy[i]
                t = i - LAG1
                if 0 <= t < T:
                    if not acb2[0]:
                        sc.wait_ge(ld_b2, LD_B2)
                        acb2[0] = True
                    sc.wait_ge(pe, pe_w1[t])
                    ainc(sc.activation(
                        out=uT[:, t * 128: (t + 1) * 128], in_=mmP[t % 2][:],
                        func=ACT.Relu, bias=b1_sb[:],
                    ))
                    assert counters["ac"] == ac_relu[t]
                t = i - LAG2
                if 0 <= t < T:
                    sc.wait_ge(pe, pe_h2[t])
                    ainc(sc.activation(
                        out=h2p[:, t * C: (t + 1) * C], in_=h2P[t % 2][:],
                        func=ACT.Copy, scale=dvo_sb[:, t: t + 1],
                    ))
                    assert counters["ac"] == ac_h2[t]
            for i in range(T + EXLAG):
                if i < T:
                    sc.wait_ge(pe, pe_blk2[i])
                    ainc(sc.activation(
                        out=qmB[:, i * C: (i + 1) * C], in_=pm2[i % 2][:],
                        func=ACT.Copy, scale=dvo_sb[:, i: i + 1],
                    ))
                    assert counters["ac"] == ac_qm[i]
                b = i - EXLAG
                if 0 <= b < T:
                    sc.wait_ge(ve, ve_negmax[b])
                    ainc(sc.activation(
                        out=qe[:], in_=qmB[:, b * C: (b + 1) * C],
                        func=ACT.Exp, bias=nmxB[:, b: b + 1],
                        accum_out=smeB[:, b: b + 1],
                    ))
                    sc.drain()
                    assert counters["ac"] == ac_exp[b]
                    if b == SPLITB - 1:
                        ainc(sc.activation(
                            out=lnsB[:, :SPLITB], in_=smeB[:, :SPLITB],
                            func=ACT.Ln,
                        ))
                        assert counters["ac"] == AC_LN1
            ainc(sc.activation(
                out=lnsB[:, SPLITB:T], in_=smeB[:, SPLITB:T], func=ACT.Ln,
            ))
            assert counters["ac"] == AC_LN2

    nc.compile()
    return nc


# ----------------------------------------------------------------------------
# Public entry point.
# ----------------------------------------------------------------------------

_CACHE = {}
LAST_RESULTS = None  # BassKernelResults from the most recent traced run
_LAST_KEY = None     # (n, cfg) of the most recent kernel() call
_LAST_MAPS = None    # per-core input maps of the most recent kernel() call


def _get_kernel(n, cfg):
    key = (n, cfg)
    if key not in _CACHE:
        _CACHE[key] = _build(n, cfg)
    return _CACHE[key]


def _in_maps(x, W1, b1, W2, b2, dinv, cores, cfg, n):
    shard, half, T, shard_pad = _shard_sizes(n)
    NBMAX = max(cfg[0], cfg[1], cfg[2] + cfg[3])
    x16 = (np.asarray(x, np.float32) * dinv[:, None]).astype(np.float16)
    xlo = np.ascontiguousarray(x16[:half])
    xhi = np.ascontiguousarray(x16[half:])
    b2r = np.tile(np.asarray(b2, np.float16)[None, :], (128, 1))
    twoI = (2.0 * np.eye(128)).astype(np.float16)
    oneN = np.full((128, 128), 1.0 / 128.0, np.float16)
    iota = np.tile(
        np.repeat(np.arange(128, dtype=np.float16), NBMAX)[None, :], (128, 1))
    maps = []
    for k in range(NCORES):
        xo = np.zeros((shard_pad, DIN), np.float16)
        xo[:shard] = x16[k * shard: (k + 1) * shard]
        dvp = np.zeros(shard_pad, np.float32)
        dvp[:shard] = dinv[k * shard: (k + 1) * shard]
        dvo = np.ascontiguousarray(dvp.reshape(T, 128).T)
        drep = np.tile(dvp.astype(np.float16)[None, :], (128, 1))
        m = dict(
            xlo=xlo, xhi=xhi, xown=xo, dinvown=dvo, dinvrep=drep,
            w1=np.asarray(W1, np.float16), w2=np.asarray(W2, np.float16),
            b1=np.asarray(b1, np.float32).reshape(H, 1), b2r=b2r, twoI=twoI,
            oneN=oneN, iota=iota,
        )
        m.update(cores[k])
        maps.append(m)
    return maps


def kernel(x, edge_index, W1, b1, W2, b2):
    n = x.shape[0]
    dinv, cores, cfg = _preprocess(edge_index, n)
    nc = _get_kernel(n, cfg)
    maps = _in_maps(x, W1, b1, W2, b2, dinv, cores, cfg, n)
    global _LAST_KEY, _LAST_MAPS
    _LAST_KEY = (n, cfg)
    _LAST_MAPS = maps

    if os.environ.get("KERNEL_SIM"):
        from concourse import bass_interp

        sim = bass_interp.MultiCoreSim(nc, NCORES, trace=True)
        for k in range(NCORES):
            for kk, vv in maps[k].items():
                sim.cores[k].tensor(kk)[:] = vv
        sim.simulate()
        print(f"sim global_time: {sim.global_time} ns")
        shard = _shard_sizes(n)[0]
        outs = [np.array(sim.cores[k].tensor("out"))[:shard]
                for k in range(NCORES)]
    else:
        global LAST_RESULTS
        trace = bool(os.environ.get("KERNEL_TRACE"))
        kwargs = {}
        if trace:
            tmpdir = os.environ.get("KERNEL_TRACE_DIR") or None
            if tmpdir:
                os.makedirs(tmpdir, exist_ok=True)
            kwargs = dict(trace=True, tmpdir=tmpdir)
        res = run_bass_kernel_spmd(nc, maps, list(range(NCORES)), **kwargs)
        if trace:
            LAST_RESULTS = res
        shard = _shard_sizes(n)[0]
        outs = [res.results[k]["out"][:shard] for k in range(NCORES)]
    return np.concatenate(outs, axis=0)

